# revision 26
# baseline (speedup 1.0000x reference)
"""Trainium2 Bass kernel for nn_Colorizer (retrieval_knn).

Computation (per reference frame r of 3, for each pixel p of a 128x128 image):
  corr[r, n, p] = <feats_t[:, p], feats_r[r, :, p + offset(n)]>   n in 13x13 window
  q_val[r, p]  = max_n corr ; q_idx[r, p] = argmax_n corr (first occurrence)
  gathered[r, c, p] = quantized_sub[r, c, p + offset(q_idx)]      (zero padded)
  out[c, p] = sum_r softmax_r(q_val)[r] * gathered[r, c, p]

Sharding: the spatial h dim is split into 8 bands of 16 rows (one per core);
each core handles all 3 refs for its band, so the softmax over refs is local
and no device collective is needed.  Host reassembles the row bands.

Device algorithm per core, per (ref, tile) pair (3 refs x 16 tiles of 16x8
pixels -> 48 pairs, processed ref-major so the first pairs only depend on
ref 0's loads -> short DMA ramp):
  * TensorE computes the Gram matrix between the tile's feats_t vectors and
    the 28x20 zero-padded feats_r halo window (560 columns over 2 PSUM banks)
    as a 3-pass fp16 split (hi*hi + hi*lo + lo*hi; the dropped lo*lo term is
    ~6e-6 which is far below the 3.3e-5 min top-2 corr gap on these inputs,
    so argmax and softmax match fp32), then accumulates a -1e30 valid-window
    mask via a bf16 identity matmul.  Eight dummy matmuls on the
    already-loaded mask warm the PE's HAM clock gate during input load.
  * Inputs are packed into few large DMAs (HWDGE issue costs ~0.6us
    each; issuing 22 separate loads used to cost 13us of ramp), all on ONE
    HWDGE queue in strict consumption order -- two queues starve each other
    (SDMA round-robin once split 56 vs 287 GB/s).  Ref 0's A-block and
    feats_t are further split so the first corr matmul starts after ~1.1MB
    has landed (~15.6us including the ~7.5us fixed NEFF preamble).
  * ScalarE (ACT) copies each masked PSUM pair to a contiguous SBUF stream.
  * VectorE computes per-group maxes (one batched tensor_reduce) and
    argmaxes (one find_index8 per group) over groups of 1,1,2,4x10,2,1,1
    pairs (small edge groups start the DVE scan stream ~2.3us earlier and
    shorten the post-last-find gather tail; 4-pair middle groups keep the
    gather stream interleaved 1:1 with the scan stream -- 8-pair groups
    made it bursty and built a backlog, and >=5-pair groups consume corr
    faster than the PE+copy pipeline produces it).  The 2 scans/pair at
    ~1 elem/cycle are the DVE's stock-ISA floor and pace the ~64us body.
  * The gather runs as one batched SWDGE dma_gather per group (measured
    ~8.4ns per gathered row of descriptor generation vs ~11ns/row for the
    old per-pair indirect DMAs, and off the critical path): the flat find
    index + group offset directly addresses a DRAM table [48*560 rows,
    64 fp32] of each (pair, window position)'s quantized pixel padded to
    256B (dma_gather's minimum elem).  dma_gather wants indices int16,
    wrapped j -> (partition j%16, col j//16) and replicated to all 8 Q7
    core groups; that layout is produced by one tiny fp32 PE matmul per
    group (a [128,128] replicated-comb permutation matrix against
    (idx + rowbase) * qh-indicator built by one DVE scalar_tensor_tensor),
    cast to int16 by ScalarE.  Gathered rows land [pixel partition, pair
    slot, 64] with the pair's 3 channels at [0:3].
  * A tiny fp32 softmax over the 3 refs weights the gathered values,
    in 4-tile quarters so only the last quarter waits on the last gather.
"""

import os

import numpy as np

import concourse.bass as bass
import concourse.mybir as mybir
import concourse.tile as tile
from concourse import bacc
from concourse.bass_utils import run_bass_kernel_spmd

F32 = mybir.dt.float32
F16 = mybir.dt.float16
BF16 = mybir.dt.bfloat16
U32 = mybir.dt.uint32
I16 = mybir.dt.int16

NCORES = 8
NREF, C, H, W = 3, 128, 128, 128
RAD = 6                      # patch radius
PS = 2 * RAD + 1             # 13
CQ = 3                       # quantized channels
SUB = 4                      # quantized_r spatial subsample stride

ROWS = H // NCORES           # 16 rows per core
XB = 8                       # x block size
NT = W // XB                 # 16 tiles per ref
WY = ROWS + 2 * RAD          # 28 window rows
WX = XB + 2 * RAD            # 20 window cols
WIN = WY * WX                # 560
HALF = WY // 2               # 14 window rows per PSUM bank
NHALF = HALF * WX            # 280 columns per matmul
PW = W + 2 * RAD             # 140 padded width
XA = 76                      # x-block A: padded cols [0, 76), tiles 0-7
XB2 = 84                     # x-block B: padded cols [56, 140), tiles 8-15
XB0 = PW - XB2               # 56, B block origin
NP = NREF * NT               # 48 (ref, tile) pairs; pair p = r*NT + t
GROW = 64                    # gather table row: 3 used + pad to 256B
_GRP_SIZES = [1, 1, 2] + [4] * 10 + [2, 1, 1]
FIND_GROUPS = []
_s = 0
for _n in _GRP_SIZES:
    FIND_GROUPS.append(list(range(_s, _s + _n)))
    _s += _n
NEG = -1.0e30
NWARM = 12

_CACHE: dict = {}


def _build_program(debug_taps: bool = False) -> bacc.Bacc:
    nc = bacc.Bacc("TRN2", target_bir_lowering=False, debug=False)

    # packed inputs (few, large DMAs)
    cpbf_d = nc.dram_tensor("cpbf", [128, WIN + 128], BF16, kind="ExternalInput")
    cpf32_d = nc.dram_tensor("cpf32", [128, 136], F32, kind="ExternalInput")
    ftp_d = nc.dram_tensor("ftp", [C, 2, NT * 128], F16, kind="ExternalInput")
    frA_d = nc.dram_tensor("frA", [NREF, C, 2 * WY * XA], F16, kind="ExternalInput")
    # ref 0's A block split into two overlapping col-blocks so the first
    # corr matmul only waits for ~0.7MB: a = padded cols [0,48) (tiles 0-3),
    # b = [28,76) (tiles 4-7)
    frA0a_d = nc.dram_tensor("frA0a", [C, 2 * WY * 48], F16, kind="ExternalInput")
    frA0b_d = nc.dram_tensor("frA0b", [C, 2 * WY * 48], F16, kind="ExternalInput")
    frB_d = nc.dram_tensor("frB", [NREF, C, 2 * WY * XB2], F16, kind="ExternalInput")
    # gather table: row p*WIN + n = quantized pixel (3ch + pad) of window
    # position n for pair p = r*NT + t
    qtable_d = nc.dram_tensor("qtable", [NP * WIN, GROW], F32, kind="ExternalInput")
    # raw layout [pixel_partition=(yl,xl), tile, channel]; host untangles
    out_d = nc.dram_tensor("out", [128, NT * CQ], F32, kind="ExternalOutput")
    if debug_taps:
        qval_d = nc.dram_tensor("qval_dbg", [128, NP], F32, kind="ExternalOutput")
        idx_d = nc.dram_tensor(
            "idx_dbg", [128, 8 * len(FIND_GROUPS)], U32, kind="ExternalOutput"
        )
        gath_d = nc.dram_tensor(
            "gath_dbg", [128, NP * GROW], F32, kind="ExternalOutput"
        )

    with tile.TileContext(nc) as tc:
        with (
            tc.tile_pool(name="const", bufs=1) as constp,
            tc.tile_pool(name="psum", bufs=3, space="PSUM") as psump,
            tc.tile_pool(name="psxf", bufs=2, space="PSUM") as psxfp,
            tc.tile_pool(name="small", bufs=1) as smallp,
        ):
            cpbf = constp.tile([128, WIN + 128], BF16, tag="cpbf")
            cpf32 = constp.tile([128, 136], F32, tag="cpf32")
            ftp = constp.tile([C, 2, NT * 128], F16, tag="ftp")
            frA_sb, frB_sb = [], []
            for r in range(NREF):
                fra_ = constp.tile([C, 2 * WY * XA], F16, tag=f"frA{r}")
                frb_ = constp.tile([C, 2 * WY * XB2], F16, tag=f"frB{r}")
                frA_sb.append(fra_)
                frB_sb.append(frb_)
            frA0a = constp.tile([C, 2 * WY * 48], F16, tag="frA0a")
            frA0b = constp.tile([C, 2 * WY * 48], F16, tag="frA0b")
            mask_sb = cpbf[:, 0:WIN]
            ident_sb = cpbf[:, WIN : WIN + 128]
            comb_sb = cpf32[:, 0:128]
            qhmask_sb = cpf32[:, 128:136]

            # one HWDGE queue, strict priority order (two queues starve each
            # other: SDMA round-robin split the sync queue to ~56GB/s while
            # the scalar queue hogged the HBM): consts -> first-needed feats
            # -> ref 0 windows -> the rest in consumption order
            nc.sync.dma_start(out=cpbf[:], in_=cpbf_d.ap())
            nc.sync.dma_start(
                out=ftp[:, :, 0:512], in_=ftp_d.ap()[:, :, 0:512]
            )
            nc.sync.dma_start(out=frA0a[:], in_=frA0a_d.ap())
            nc.sync.dma_start(out=cpf32[:], in_=cpf32_d.ap())
            nc.sync.dma_start(
                out=ftp[:, :, 512:1024], in_=ftp_d.ap()[:, :, 512:1024]
            )
            nc.sync.dma_start(out=frA0b[:], in_=frA0b_d.ap())
            nc.sync.dma_start(out=frB_sb[0][:], in_=frB_d.ap()[0])
            nc.sync.dma_start(
                out=ftp[:, :, 1024:], in_=ftp_d.ap()[:, :, 1024:]
            )
            for r in range(1, NREF):
                nc.sync.dma_start(out=frA_sb[r][:], in_=frA_d.ap()[r])
                nc.sync.dma_start(out=frB_sb[r][:], in_=frB_d.ap()[r])

            # warm the PE's HAM clock gate on the already-loaded mask/ident
            warm_ps = psump.tile([128, 1024], F32, tag="ps")
            for _ in range(NWARM):
                nc.tensor.matmul(
                    warm_ps[:, 0:512], ident_sb, mask_sb[:, 0:512],
                    start=True, stop=True,
                )

            # corr stream: 3 rotating regions of up to 8 pairs x 560 fp32
            corr_sb = smallp.tile([128, 3, 8, WIN], F32, tag="corr")
            qval = smallp.tile([128, NP], F32, tag="qval")
            idx = smallp.tile([128, 8 * len(FIND_GROUPS)], U32, tag="idx")
            # gathered rows: [pixel partition, pair slot, 3ch + pad]
            gath = smallp.tile([128, NP, GROW], F32, tag="gath")
            # qval is read in 8-wide in_max windows before all slots are
            # written; zero-init so the garbage needles are benign
            nc.gpsimd.memset(qval[:], 0.0)

            grp_of = {}
            for g, grp in enumerate(FIND_GROUPS):
                for p in grp:
                    grp_of[p] = g

            for r in range(NREF):
                for t in range(NT):
                    p = r * NT + t
                    g = grp_of[p]
                    grp = FIND_GROUPS[g]
                    g0 = grp[0]
                    j = p - g0
                    lhs_hi = ftp[:, 0, t * 128 : (t + 1) * 128]
                    lhs_lo = ftp[:, 1, t * 128 : (t + 1) * 128]
                    if t < NT // 2:
                        if r == 0 and t < 4:
                            fv = frA0a[:].rearrange(
                                "c (s y x) -> c s y x", s=2, x=48
                            )
                            x0 = t * XB
                        elif r == 0:
                            fv = frA0b[:].rearrange(
                                "c (s y x) -> c s y x", s=2, x=48
                            )
                            x0 = t * XB - 28
                        else:
                            fv = frA_sb[r][:].rearrange(
                                "c (s y x) -> c s y x", s=2, x=XA
                            )
                            x0 = t * XB
                    else:
                        fv = frB_sb[r][:].rearrange(
                            "c (s y x) -> c s y x", s=2, x=XB2
                        )
                        x0 = t * XB - XB0
                    rh1 = fv[:, 0, 0:HALF, x0 : x0 + WX]
                    rh2 = fv[:, 0, HALF:WY, x0 : x0 + WX]
                    rl1 = fv[:, 1, 0:HALF, x0 : x0 + WX]
                    rl2 = fv[:, 1, HALF:WY, x0 : x0 + WX]
                    ps = psump.tile([128, 1024], F32, tag="ps")
                    ps1 = ps[:, 0:NHALF]
                    ps2 = ps[:, 512 : 512 + NHALF]
                    # 3-pass fp16 split ordered for stationary reuse
                    nc.tensor.matmul(ps1, lhs_hi, rh1, start=True, stop=False)
                    nc.tensor.matmul(ps2, lhs_hi, rh2, start=True, stop=False)
                    nc.tensor.matmul(ps1, lhs_hi, rl1, start=False, stop=False)
                    nc.tensor.matmul(ps2, lhs_hi, rl2, start=False, stop=False)
                    nc.tensor.matmul(ps1, lhs_lo, rh1, start=False, stop=False)
                    nc.tensor.matmul(ps2, lhs_lo, rh2, start=False, stop=False)
                    # valid-window mask (-1e30 outside own 13x13 patch)
                    nc.tensor.matmul(
                        ps1, ident_sb, mask_sb[:, 0:NHALF],
                        start=False, stop=True,
                    )
                    nc.tensor.matmul(
                        ps2, ident_sb, mask_sb[:, NHALF:WIN],
                        start=False, stop=True,
                    )
                    # ACT drains PSUM into the batched SBUF stream
                    psv = ps[:].rearrange("p (b n) -> p b n", b=2)[:, :, 0:NHALF]
                    nc.scalar.copy(
                        out=corr_sb[:, g % 3, j].rearrange(
                            "p (b n) -> p b n", b=2
                        ),
                        in_=psv,
                    )
                    if p == grp[-1]:
                        gl = len(grp)
                        reg = corr_sb[:, g % 3, 0:gl]
                        # per-pair max over the group's pairs
                        nc.vector.tensor_reduce(
                            out=qval[:, g0 : g0 + gl],
                            in_=reg,
                            axis=mybir.AxisListType.X,
                            op=mybir.AluOpType.max,
                        )
                        # argmax: one find_index8 over the group stream;
                        # pair p's index lands in slot 8g + (p - w0)
                        w0 = min(g0, NP - 8)
                        nc.vector.max_index(
                            out=idx[:, 8 * g : 8 * g + 8],
                            in_max=qval[:, w0 : w0 + 8],
                            in_values=reg.rearrange("p a b -> p (a b)"),
                        )
                        # build the wrapped int16 row-index layout for
                        # dma_gather: masked[q, (jg, qh)] =
                        #   (idx[q, jg] + g0*WIN) * (q//16 == qh)
                        mskd = smallp.tile([128, 8, 8], F32, tag=f"mskd{g}")
                        s0 = 8 * g + (g0 - w0)
                        iv = idx[:, s0 : s0 + gl].rearrange(
                            "p (n o) -> p n o", o=1
                        ).to_broadcast([128, gl, 8])
                        qm = qhmask_sb.rearrange(
                            "p (o w) -> p o w", o=1
                        ).to_broadcast([128, gl, 8])
                        nc.vector.scalar_tensor_tensor(
                            out=mskd[:, 0:gl],
                            in0=iv,
                            scalar=float(g0 * WIN),
                            in1=qm,
                            op0=mybir.AluOpType.add,
                            op1=mybir.AluOpType.mult,
                        )
                        # comb^T @ mskd: row j%16, col j//16, all 8 replicas
                        xf = psxfp.tile([128, 64], F32, tag="xf")
                        nc.tensor.matmul(
                            xf[:, 0 : gl * 8],
                            comb_sb,
                            mskd[:, 0:gl].rearrange("p a b -> p (a b)"),
                            start=True, stop=True,
                        )
                        idx16 = smallp.tile([128, 64], I16, tag=f"idx16_{g}")
                        nc.scalar.copy(
                            out=idx16[:, 0 : gl * 8], in_=xf[:, 0 : gl * 8]
                        )
                        # one batched gather for the whole group: row
                        # j=(slot, pixel) -> gath[pixel, g0+slot, :]
                        nc.gpsimd.dma_gather(
                            out_ap=gath[:, g0 : g0 + gl],
                            in_ap=qtable_d.ap(),
                            idxs_ap=idx16[:, 0 : gl * 8],
                            num_idxs=gl * 128,
                            num_idxs_reg=gl * 128,
                            elem_size=GROW,
                        )

            if debug_taps:
                nc.sync.dma_start(out=qval_d.ap(), in_=qval[:])
                nc.sync.dma_start(out=idx_d.ap(), in_=idx[:])
                nc.sync.dma_start(
                    out=gath_d.ap(), in_=gath[:].rearrange("p a b -> p (a b)")
                )

            # softmax + weighted combine in 4-tile quarters: quarter k only
            # depends on pairs {r*NT + t : t < 4k+4}, so earlier quarters
            # overlap the later groups' gathers
            for k in range(4):
                _softmax_combine_quarter(
                    nc, smallp, qval, gath, out_d, k, 4 * k, 4
                )

    nc.compile()
    return nc


def _softmax_combine_quarter(nc, smallp, qval, gath, out_d, h, t0, tn):
    # qval [128, (r, t)]; gath [128, (r, t), GROW]
    qvv = qval[:].rearrange("p (r t) -> p r t", r=NREF)[:, :, t0 : t0 + tn]
    qv = [qvv[:, r] for r in range(NREF)]
    m01 = smallp.tile([128, tn], F32, tag=f"m01_{h}")
    nc.vector.tensor_tensor(
        out=m01[:], in0=qv[0], in1=qv[1], op=mybir.AluOpType.max
    )
    mm = smallp.tile([128, tn], F32, tag=f"mm_{h}")
    nc.vector.tensor_tensor(
        out=mm[:], in0=m01[:], in1=qv[2], op=mybir.AluOpType.max
    )
    es = []
    for r in range(NREF):
        e_ = smallp.tile([128, tn], F32, tag=f"e{r}_{h}")
        nc.vector.tensor_tensor(
            out=e_[:], in0=qv[r], in1=mm[:], op=mybir.AluOpType.subtract
        )
        nc.scalar.activation(
            out=e_[:], in_=e_[:], func=mybir.ActivationFunctionType.Exp
        )
        es.append(e_)
    ssum = smallp.tile([128, tn], F32, tag=f"ssum_{h}")
    nc.vector.tensor_tensor(
        out=ssum[:], in0=es[0][:], in1=es[1][:], op=mybir.AluOpType.add
    )
    nc.vector.tensor_tensor(
        out=ssum[:], in0=ssum[:], in1=es[2][:], op=mybir.AluOpType.add
    )
    rec = smallp.tile([128, tn], F32, tag=f"rec_{h}")
    nc.vector.reciprocal(out=rec[:], in_=ssum[:])

    gv = gath[:].rearrange("p (r t) w -> p r t w", r=NREF)[
        :, :, t0 : t0 + tn, 0:CQ
    ]
    oacc = smallp.tile([128, tn * CQ], F32, tag=f"oacc_{h}")
    oaccv = oacc[:].rearrange("p (s c) -> p s c", c=CQ)
    for r in range(NREF):
        w_ = smallp.tile([128, tn], F32, tag=f"w{r}_{h}")
        nc.vector.tensor_tensor(
            out=w_[:], in0=es[r][:], in1=rec[:], op=mybir.AluOpType.mult
        )
        wb = w_[:].rearrange("p (s o) -> p s o", o=1).to_broadcast([128, tn, CQ])
        if r == 0:
            nc.vector.tensor_tensor(
                out=oaccv, in0=gv[:, r], in1=wb, op=mybir.AluOpType.mult
            )
        else:
            term = smallp.tile([128, tn * CQ], F32, tag=f"term{r}_{h}")
            termv = term[:].rearrange("p (s c) -> p s c", c=CQ)
            nc.vector.tensor_tensor(
                out=termv, in0=gv[:, r], in1=wb, op=mybir.AluOpType.mult
            )
            nc.vector.tensor_tensor(
                out=oaccv, in0=oaccv, in1=termv, op=mybir.AluOpType.add
            )

    nc.sync.dma_start(
        out=out_d.ap()[:, t0 * CQ : (t0 + tn) * CQ], in_=oacc[:]
    )


def _host_prep(feats_r, feats_t, quantized_r):
    """Build the 8 per-core input maps."""
    import ml_dtypes

    frp_full = np.zeros((NREF, C, H + 2 * RAD, PW), np.float32)
    frp_full[:, :, RAD : RAD + H, RAD : RAD + W] = feats_r[:, 0]
    frh_full = frp_full.astype(np.float16)
    frl_full = (frp_full - frh_full.astype(np.float32)).astype(np.float16)

    def blocks(y0):
        # hi|lo packed per ref: A [NREF, C, 2*WY*XA], B [NREF, C, 2*WY*XB2]
        bh = frh_full[:, :, y0 : y0 + WY, :]
        bl = frl_full[:, :, y0 : y0 + WY, :]
        A = np.concatenate(
            [
                bh[..., 0:XA].reshape(NREF, C, WY * XA),
                bl[..., 0:XA].reshape(NREF, C, WY * XA),
            ],
            axis=2,
        )
        B = np.concatenate(
            [
                bh[..., XB0:PW].reshape(NREF, C, WY * XB2),
                bl[..., XB0:PW].reshape(NREF, C, WY * XB2),
            ],
            axis=2,
        )
        return np.ascontiguousarray(A), np.ascontiguousarray(B)

    ft = feats_t[0]
    fth = ft.astype(np.float16)
    ftl = (ft - fth.astype(np.float32)).astype(np.float16)

    qr = np.ascontiguousarray(quantized_r[:, 0, :, ::SUB, ::SUB], np.float32)
    qrp_full = np.zeros((NREF, H + 2 * RAD, PW, CQ), np.float32)
    qrp_full[:, RAD : RAD + H, RAD : RAD + W, :] = qr.transpose(0, 2, 3, 1)

    # mask[p=(yl,xl), n=(y',x')] = 0 inside pixel (yl,xl)'s own 13x13 patch
    yl = np.arange(ROWS)[:, None, None, None]
    xl = np.arange(XB)[None, :, None, None]
    yw = np.arange(WY)[None, None, :, None]
    xw = np.arange(WX)[None, None, None, :]
    valid = (
        (yw - yl >= 0) & (yw - yl < PS) & (xw - xl >= 0) & (xw - xl < PS)
    )
    mask = np.where(valid, 0.0, NEG).astype(np.float32).reshape(128, WIN)
    ident = np.eye(128, dtype=np.float32)
    cpbf = np.concatenate([mask, ident], axis=1).astype(ml_dtypes.bfloat16)

    qi = np.arange(128)
    comb = (qi[:, None] % 16 == qi[None, :] % 16).astype(np.float32)
    qhmask = (qi[:, None] // 16 == np.arange(8)[None, :]).astype(np.float32)
    cpf32 = np.concatenate([comb, qhmask], axis=1)

    def ft_layout(a):
        # [c, yl, t, xl] -> [c, t, yl, xl]: tile-major, pixels contiguous
        return np.ascontiguousarray(
            a.reshape(C, ROWS, NT, XB)
            .transpose(0, 2, 1, 3)
            .reshape(C, ROWS * W)
        )

    in_maps = []
    for k in range(NCORES):
        y0 = ROWS * k
        frA, frB = blocks(y0)
        ftp = np.stack(
            [
                ft_layout(fth[:, y0 : y0 + ROWS, :]),
                ft_layout(ftl[:, y0 : y0 + ROWS, :]),
            ],
            axis=1,
        )
        bh0 = frh_full[0, :, y0 : y0 + WY, :]
        bl0 = frl_full[0, :, y0 : y0 + WY, :]

        def ab(c0):
            return np.ascontiguousarray(np.concatenate(
                [
                    bh0[..., c0 : c0 + 48].reshape(C, WY * 48),
                    bl0[..., c0 : c0 + 48].reshape(C, WY * 48),
                ],
                axis=1,
            ))

        # dict order = staging order: put the earliest-read tensors
        # first so a lagging host->DRAM upload can't race the kernel's
        # early reads (gathers hit qtable from ~t=22us)
        m = {
            "cpbf": cpbf,
            "ftp": np.ascontiguousarray(ftp),
            "frA0a": ab(0),
            "cpf32": cpf32,
            "frA0b": ab(28),
            "frB": frB,
            "frA": frA,
        }
        # gather table [NP*WIN, GROW]: row p*WIN + wy*WX + wx = quantized
        # pixel (3ch) of window position (wy, wx) for pair p = r*NT + t
        qc = qrp_full[:, y0 : y0 + WY, :, :]  # [NREF, WY, PW, CQ]
        tbl = np.zeros((NREF, NT, WY, WX, GROW), np.float32)
        for t in range(NT):
            tbl[:, t, :, :, 0:CQ] = qc[:, :, t * XB : t * XB + WX, :]
        m["qtable"] = tbl.reshape(NP * WIN, GROW)
        # re-insert so qtable stages before the late-read ref 1/2 windows
        m["frA"] = m.pop("frA")
        m["frB"] = m.pop("frB")
        in_maps.append(m)
    return in_maps


def _install_ntff_shim():
    """This container's antenv lacks axon_hooks, so run_bass_kernel_spmd's
    trace path can't find the NTFF profile hook. Inject the module and
    register the ctypes-based hook from the boot script. Best-effort."""
    try:
        import sys
        import types

        if "antenv.axon_hooks" in sys.modules:
            return
        mod = types.ModuleType("antenv.axon_hooks")
        holder = [None]
        mod.set_axon_ntff_profile_hook = lambda h: holder.__setitem__(0, h)
        mod.get_axon_ntff_profile_hook = lambda: holder[0]
        sys.modules["antenv.axon_hooks"] = mod
        import antenv

        antenv.axon_hooks = mod
        from trn_agent_boot.trn_boot import _ntff_profile_via_ctypes

        hook = _ntff_profile_via_ctypes("/opt/axon/libaxon_pjrt.so")
        if hook is not None:
            mod.set_axon_ntff_profile_hook(hook)
    except Exception as e:  # pragma: no cover - tracing is best-effort
        print(f"ntff shim install failed: {e}")


last_exec_time_ns = None


def kernel(feats_r, feats_t, quantized_r, ref_index=None, current_ind=None):
    global last_exec_time_ns
    feats_r = np.asarray(feats_r, np.float32)
    feats_t = np.asarray(feats_t, np.float32)
    quantized_r = np.asarray(quantized_r, np.float32)

    in_maps = _host_prep(feats_r, feats_t, quantized_r)

    if "nc" not in _CACHE:
        _CACHE["nc"] = _build_program()
    nc = _CACHE["nc"]

    trace = bool(int(os.environ.get("KERNEL_TRACE", "0")))
    kwargs = {}
    if trace:
        _install_ntff_shim()
        tdir = os.environ.get("KERNEL_TRACE_DIR")
        if tdir:
            os.makedirs(tdir, exist_ok=True)
            kwargs["tmpdir"] = tdir
    res = run_bass_kernel_spmd(
        nc, in_maps, list(range(NCORES)), trace=trace, **kwargs
    )
    last_exec_time_ns = res.exec_time_ns

    out = np.concatenate(
        [_unshard_core(res.results[k]["out"]) for k in range(NCORES)], axis=1
    )
    return np.ascontiguousarray(out.reshape(1, CQ, H, W), np.float32)


def _unshard_core(raw):
    # raw [128, NT*CQ] with partition p=(yl,xl), free (t, c) -> [CQ, ROWS, W]
    r = np.asarray(raw).reshape(ROWS, XB, NT, CQ)
    return r.transpose(3, 0, 2, 1).reshape(CQ, ROWS, W)
